# revision 24
# baseline (speedup 1.0000x reference)
"""Trainium2 Bass kernel for the K_MOTE time-feature MoE routing module.

Strategy (pure data parallel, 8 cores, 16384 rows/core):
  - Host prep: aux transposed per shard; all per-partition affine constants
    (freqs / knots / centers / wavelet scales) packed into [80,1] scale/bias
    vectors; expert weights packed block-diagonally into one [98,512] matrix;
    router weights+bias packed into [98,4] (rows 80:97 = [t; aux], row 97 = ones).
  - On core: X [98, 16384] holds, per batch column, all 80 basis-feature values
    (rows 0:80, computed by ACT from a partition-broadcast copy of t using
    per-partition scale/bias), plus t/auxT/ones rows for the router.
  - Router logits for ALL 128 row-subtiles accumulate into one PSUM bank via
    K=98 N=4 matmuls; softmax + top-2 masking run whole-chunk batched on DVE.
  - Main embedding: per 128-row subtile, one K=98 N=512 fp32 matmul
    (lhsT = X column slice) -> PSUM [128,512]; eviction to SBUF fuses the
    top-2 dispatch scaling (tensor_tensor with e-major broadcast AP); big
    batched DMAs store contiguous [1024,512] row groups.
"""

import numpy as np
from contextlib import ExitStack

B, AUX, E, D, G, KF, KTOP = 131072, 16, 4, 128, 16, 16, 2
NCORES = 8
RPC = B // NCORES          # rows per core
NSUB = RPC // 128          # 128-row subtiles per core
NCHUNK = 4                 # feature/softmax pipeline chunks
CHW = RPC // NCHUNK        # chunk width (columns of X)
SPC = NSUB // NCHUNK       # subtiles per chunk
GRP = 8                    # subtiles per output DMA group

_PROG = {}


def _patch_tile_drain():
    """This container's walrus rejects >2 sync-waits on one Drain; re-emit the
    tile epilogue drain's waits as individual wait_ge instructions."""
    import concourse.tile as tile_mod
    from concourse.tile import TileContext
    if getattr(TileContext, "_drain_patched", False):
        return

    def _patched(self, tick_clock, wait_clock):
        ScopedClock = tile_mod.ScopedClock
        nc = self.nc
        probe = nc.sync.drain()
        wait_clock.add_sem_waits(probe.ins, ScopedClock({None: tick_clock.global_clock}))
        waits = list(probe.ins.sync_info.on_wait)
        if len(waits) > 2:
            name_to_handle = {h.name: h for h in self.sems.allocated().values()}
            si = probe.ins.sync_info
            si.on_wait = []
            probe.ins.sync_info = si
            for w in waits:
                nc.sync.wait_ge(name_to_handle[w.ant_name], w.wait_value)
            nc.sync.drain()
        nc.all_engine_barrier()
        popped = nc._tile_sem_poison_stack.pop()
        assert popped is self._sem_poison
        nc.clear_and_free_semaphores(list(self.sems.allocated().values()))
        nc.all_engine_barrier()

    TileContext._drain_and_barrier = _patched
    TileContext._drain_patched = True


def _split_sync_waits(nc, limit=1):
    """walrus in this container rejects instructions carrying >2 sync waits;
    hoist the overflow onto standalone EventSemaphore instructions."""
    import concourse.mybir as mybir
    n_split = 0
    for fn in nc.m.functions:
        for bb in fn.blocks:
            new = []
            for inst in bb.instructions:
                si = inst.sync_info
                if si is not None and si.on_wait and len(si.on_wait) > limit:
                    waits = list(si.on_wait)
                    keep, over = waits[-limit:], waits[:-limit]
                    for k in range(0, len(over), limit):
                        w = mybir.InstEventSemaphore(
                            name=f"{inst.name}-w{k}",
                            engine=inst.engine,
                            ins=[], outs=[],
                            sync_info=mybir.SyncInfo(
                                on_wait=over[k:k + limit], on_update=[]),
                        )
                        new.append(w)
                        n_split += 1
                    si.on_wait = keep
                    inst.sync_info = si
                new.append(inst)
            bb.instructions = new
    return n_split


def _build_program():
    if "nc" in _PROG:
        return _PROG["nc"]
    _patch_tile_drain()
    import concourse.bass as bass
    import concourse.mybir as mybir
    from concourse.tile import TileContext

    fp32 = mybir.dt.float32
    u8 = mybir.dt.uint8
    AF = mybir.ActivationFunctionType
    OP = mybir.AluOpType
    PI_2 = float(np.pi / 2)

    nc = bass.Bass()
    t_d = nc.declare_dram_parameter("t_row", [1, RPC], fp32, isOutput=False)
    auxT_d = nc.declare_dram_parameter("auxT", [AUX + 1, RPC], fp32, isOutput=False)
    cvec_d = nc.declare_dram_parameter("cvec", [112, 1], fp32, isOutput=False)
    dvec_d = nc.declare_dram_parameter("dvec", [112, 1], fp32, isOutput=False)
    wmain_d = nc.declare_dram_parameter("wmain", [112, 512], fp32, isOutput=False)
    wr_d = nc.declare_dram_parameter("wr", [18, 4], fp32, isOutput=False)
    emb_d = nc.declare_dram_parameter("emb", [RPC, 512], fp32, isOutput=True)
    rw_d = nc.declare_dram_parameter("rw", [128, NSUB * 4], fp32, isOutput=True)
    mask_d = nc.declare_dram_parameter("mask", [128, NSUB * 4], u8, isOutput=True)

    with TileContext(nc) as tc, ExitStack() as ctx:
        const = ctx.enter_context(tc.tile_pool(name="const", bufs=1))
        xpool = ctx.enter_context(tc.tile_pool(name="x", bufs=1))
        spool = ctx.enter_context(tc.tile_pool(name="s", bufs=1))
        rpool = ctx.enter_context(tc.tile_pool(name="r", bufs=2))
        opool = ctx.enter_context(tc.tile_pool(name="o", bufs=3))
        lpsum = ctx.enter_context(tc.tile_pool(name="lp", bufs=2, space="PSUM"))
        mpsum = ctx.enter_context(tc.tile_pool(name="mp", bufs=6, space="PSUM"))

        cvec = const.tile([112, 1], fp32)
        dvec = const.tile([112, 1], fp32)
        wmain = const.tile([112, 512], fp32)
        wr_full = const.tile([82, 4], fp32)
        zro = const.tile([128, 1], fp32)
        onec = const.tile([128, 1], fp32)
        magic = const.tile([128, 1], fp32)
        nc.sync.dma_start(out=cvec[:], in_=cvec_d[:])
        nc.sync.dma_start(out=dvec[:], in_=dvec_d[:])
        nc.sync.dma_start(out=wmain[:], in_=wmain_d[:])
        nc.sync.dma_start(out=wr_full[64:82, :], in_=wr_d[:])
        nc.gpsimd.memset(zro[:], 0.0)
        nc.gpsimd.memset(onec[:], 1.0)
        nc.gpsimd.memset(magic[:], 12582912.0)  # 1.5 * 2^23: fp32 round-to-int

        X = xpool.tile([112, RPC], fp32)

        for c in range(NCHUNK):
            c0, c1 = c * CHW, (c + 1) * CHW
            # broadcast t across partitions 0:81 (feature inputs + router t row)
            nc.sync.dma_start(
                out=X[0:112, c0:c1],
                in_=t_d[0:1, c0:c1].partition_broadcast(112),
            )
            nc.sync.dma_start(out=X[65:82, c0:c1], in_=auxT_d[:, c0:c1])

            # ---- router logits (rows 96:114 only; independent of features)
            psl = lpsum.tile([128, 4 * SPC], fp32, tag="logits")
            for j in range(SPC):
                s = c * SPC + j
                nc.tensor.matmul(
                    psl[:, 4 * j:4 * j + 4],
                    lhsT=X[64:82, 128 * s:128 * (s + 1)], rhs=wr_full[64:82, :],
                    start=True, stop=True, skip_group_check=True,
                )

            S = spool.tile([112, CHW], fp32, tag="sq")
            V = spool.tile([112, CHW], fp32, tag="wv")

            # trig-table ops first, then exp-table ops (2 table sets per chunk)
            # fourier rows 0:32: y = f*t/2pi (+0.25 for cos rows); range-reduce
            # r = y - round(y) with the fp32 magic-add trick, then sin(2pi r).
            Yr = spool.tile([32, CHW], fp32, tag="yr")
            Kr = spool.tile([32, CHW], fp32, tag="kr")
            nc.scalar.activation(Yr[:, :], X[0:32, c0:c1], AF.Identity,
                                 bias=dvec[0:32], scale=cvec[0:32])
            # Kr = y + M (exact fp32 RNE on DVE); then Yr := (Kr - M) - y = -r
            nc.vector.tensor_scalar_add(Kr[:, :], Yr[:, :], 12582912.0)
            nc.vector.scalar_tensor_tensor(Yr[:, :], Kr[:, :], -12582912.0,
                                           Yr[:, :], op0=OP.add, op1=OP.subtract)
            nc.scalar.activation(X[0:32, c0:c1], Yr[:, :], AF.Sin,
                                 bias=zro[0:32], scale=float(-2 * np.pi))
            nc.scalar.activation(S[32:64, :], X[32:64, c0:c1], AF.Square,
                                 bias=dvec[32:64], scale=cvec[32:64])
            nc.scalar.activation(S[96:112, :], X[96:112, c0:c1], AF.Square,
                                 bias=dvec[96:112], scale=cvec[96:112])
            nc.scalar.activation(X[32:64, c0:c1], S[32:64, :], AF.Exp,
                                 bias=zro[32:64], scale=-1.0)
            nc.scalar.activation(X[96:112, c0:c1], S[96:112, :], AF.Exp,
                                 bias=zro[96:112], scale=-1.0)
            # wavelet amplitude (1 - u^2) = 1 - 2*q' (q' = u^2/2 in S[96:112])
            nc.scalar.activation(V[96:112, :], S[96:112, :], AF.Identity,
                                 bias=onec[96:112], scale=-2.0)
            nc.vector.tensor_mul(X[96:112, c0:c1], X[96:112, c0:c1], V[96:112, :])

            # ---- batched softmax + top-2 over [128, (j=32, e=4)]
            W2 = rpool.tile([128, 4 * SPC], fp32, tag="w2")
            Ds = rpool.tile([128, SPC], fp32, tag="ds")
            Rc = rpool.tile([128, SPC], fp32, tag="rc")
            G1 = rpool.tile([128, 4 * SPC], fp32, tag="g1")
            WL = rpool.tile([128, 4 * SPC], fp32, tag="wl")
            MF = rpool.tile([128, 4 * SPC], fp32, tag="mf")
            DSP = rpool.tile([128, 4 * SPC], fp32, tag="dsp")
            MU = rpool.tile([128, 4 * SPC], u8, tag="mu")

            w2_3 = W2[:, :].rearrange("p (j e) -> p j e", e=4)
            nc.scalar.activation(W2[:, :], psl[:, :], AF.Exp, bias=0.0, scale=1.0)
            nc.vector.reduce_sum(Ds[:, :], w2_3, axis=mybir.AxisListType.X)
            nc.vector.reciprocal(Rc[:, :], Ds[:, :])
            nc.vector.tensor_mul(w2_3, w2_3, Rc[:, :].broadcast_to((128, SPC, 4)))
            # m1 = rowmax; G1 = (w >= m1); WL = w - BIG*G1; m2 = rowmax(WL)
            nc.vector.reduce_max(Ds[:, :], w2_3, axis=mybir.AxisListType.X)
            nc.vector.tensor_tensor(G1[:, :].rearrange("p (j e) -> p j e", e=4),
                                    w2_3, Ds[:, :].broadcast_to((128, SPC, 4)),
                                    op=OP.is_ge)
            nc.vector.scalar_tensor_tensor(WL[:, :], G1[:, :], -1000.0, W2[:, :],
                                           op0=OP.mult, op1=OP.add)
            nc.vector.reduce_max(Ds[:, :], WL[:, :].rearrange("p (j e) -> p j e", e=4),
                                 axis=mybir.AxisListType.X)
            nc.vector.tensor_tensor(MF[:, :].rearrange("p (j e) -> p j e", e=4),
                                    w2_3, Ds[:, :].broadcast_to((128, SPC, 4)),
                                    op=OP.is_ge)
            nc.vector.tensor_mul(DSP[:, :], W2[:, :], MF[:, :])
            nc.vector.tensor_copy(MU[:, :], MF[:, :])

            # store raw_weights + mask for this chunk
            nc.sync.dma_start(
                out=rw_d[:, 4 * SPC * c:4 * SPC * (c + 1)], in_=W2[:, :])
            nc.sync.dma_start(
                out=mask_d[:, 4 * SPC * c:4 * SPC * (c + 1)], in_=MU[:, :])

            # ---- main expert matmuls + dispatch-scaled eviction + store
            for g in range(SPC // GRP):
                OUT = opool.tile([128, GRP * 512], fp32, tag="out")
                for jj in range(GRP):
                    j = g * GRP + jj
                    s = c * SPC + j
                    psm = mpsum.tile([128, 512], fp32, tag="mm")
                    nc.tensor.matmul(
                        psm[:, :],
                        lhsT=X[:, 128 * s:128 * (s + 1)], rhs=wmain[:],
                        start=True, stop=True,
                    )
                    nc.vector.tensor_tensor(
                        OUT[:, 512 * jj:512 * (jj + 1)].rearrange(
                            "p (e d) -> p e d", e=4),
                        psm[:, :].rearrange("p (e d) -> p e d", e=4),
                        DSP[:, 4 * j:4 * j + 4].broadcast_to((128, 4, 128)),
                        op=OP.mult,
                    )
                r0 = (c * SPC + g * GRP) * 128
                r1 = r0 + GRP * 128
                nc.sync.dma_start(
                    out=emb_d[r0:r1, :].rearrange("(j p) d -> p j d", p=128),
                    in_=OUT[:, :].rearrange("p (j d) -> p j d", d=512),
                )

    _split_sync_waits(nc)
    _PROG["nc"] = nc
    return nc


def _prep_inputs(inputs):
    ts = np.asarray(inputs["timestamp"], np.float32).reshape(B)
    aux = np.asarray(inputs["aux"], np.float32)
    router_W = np.asarray(inputs["router_W"], np.float32)
    router_b = np.asarray(inputs["router_b"], np.float32)
    freqs = np.asarray(inputs["freqs"], np.float32)
    fourier_W = np.asarray(inputs["fourier_W"], np.float32)
    knots = np.asarray(inputs["knots"], np.float32)
    spline_W = np.asarray(inputs["spline_W"], np.float32)
    centers = np.asarray(inputs["centers"], np.float32)
    gauss_W = np.asarray(inputs["gauss_W"], np.float32)
    wav_centers = np.asarray(inputs["wav_centers"], np.float32)
    wav_scales = np.asarray(inputs["wav_scales"], np.float32)
    wavelet_W = np.asarray(inputs["wavelet_W"], np.float32)

    h = 4.0 / (G - 1)
    isq2 = 1.0 / np.sqrt(2.0, dtype=np.float64)
    inv2pi = 1.0 / (2.0 * np.pi)
    cvec = np.zeros((112, 1), np.float32)
    dvec = np.zeros((112, 1), np.float32)
    cvec[0:16, 0] = freqs * inv2pi
    cvec[16:32, 0] = freqs * inv2pi
    dvec[16:32, 0] = 0.25
    cvec[32:48, 0] = 1.0 / h
    dvec[32:48, 0] = -knots / h
    cvec[48:64, 0] = 1.0
    dvec[48:64, 0] = -centers
    cvec[96:112, 0] = 1.0 / (wav_scales * np.sqrt(2.0))
    dvec[96:112, 0] = -wav_centers / (wav_scales * np.sqrt(2.0))

    wmain = np.zeros((112, 512), np.float32)
    wmain[0:32, 0:128] = fourier_W
    wmain[32:48, 128:256] = spline_W
    wmain[48:64, 256:384] = gauss_W
    wmain[96:112, 384:512] = wavelet_W
    wr = np.zeros((18, 4), np.float32)
    wr[0] = router_W[0]
    wr[1:17] = router_W[1:]
    wr[17] = router_b

    in_maps = []
    for i in range(NCORES):
        sl = slice(i * RPC, (i + 1) * RPC)
        in_maps.append({
            "t_row": np.ascontiguousarray(ts[sl]).reshape(1, RPC),
            "auxT": np.ascontiguousarray(np.vstack([aux[sl].T, np.ones((1, RPC), np.float32)])),
            "cvec": cvec, "dvec": dvec, "wmain": wmain, "wr": wr,
        })
    return in_maps


def _run(inputs, trace=False, **kw):
    from concourse import bass_utils
    nc = _build_program()
    in_maps = _prep_inputs(inputs)
    res = bass_utils.run_bass_kernel_spmd(
        nc, in_maps, core_ids=list(range(NCORES)), trace=trace, **kw)
    emb = np.concatenate([res.results[i]["emb"] for i in range(NCORES)], axis=0)

    def _untr(a, dtype):
        # [128, NSUB*4] laid out [p][s,e] -> [RPC, 4] rows b = 128*s + p
        return np.ascontiguousarray(
            a.reshape(128, NSUB, 4).transpose(1, 0, 2).reshape(RPC, 4)).astype(dtype)

    rw = np.concatenate(
        [_untr(res.results[i]["rw"], np.float32) for i in range(NCORES)], axis=0)
    mask = np.concatenate(
        [_untr(res.results[i]["mask"], np.uint8) for i in range(NCORES)],
        axis=0).astype(bool)
    return (emb, rw, mask), res


def kernel(**inputs):
    out, _ = _run(inputs, trace=False)
    return out


# revision 25
# speedup vs baseline: 1.1967x; 1.1967x over previous
"""Trainium2 Bass kernel for the K_MOTE time-feature MoE routing module.

Strategy (pure data parallel, 8 cores, 16384 rows/core):
  - Host prep: aux transposed per shard; all per-partition affine constants
    (freqs / knots / centers / wavelet scales) packed into [80,1] scale/bias
    vectors; expert weights packed block-diagonally into one [98,512] matrix;
    router weights+bias packed into [98,4] (rows 80:97 = [t; aux], row 97 = ones).
  - On core: X [98, 16384] holds, per batch column, all 80 basis-feature values
    (rows 0:80, computed by ACT from a partition-broadcast copy of t using
    per-partition scale/bias), plus t/auxT/ones rows for the router.
  - Router logits for ALL 128 row-subtiles accumulate into one PSUM bank via
    K=98 N=4 matmuls; softmax + top-2 masking run whole-chunk batched on DVE.
  - Main embedding: per 128-row subtile, one K=98 N=512 fp32 matmul
    (lhsT = X column slice) -> PSUM [128,512]; eviction to SBUF fuses the
    top-2 dispatch scaling (tensor_tensor with e-major broadcast AP); big
    batched DMAs store contiguous [1024,512] row groups.
"""

import numpy as np
from contextlib import ExitStack

B, AUX, E, D, G, KF, KTOP = 131072, 16, 4, 128, 16, 16, 2
NCORES = 8
RPC = B // NCORES          # rows per core
NSUB = RPC // 128          # 128-row subtiles per core
NCHUNK = 4                 # feature/softmax pipeline chunks
CHW = RPC // NCHUNK        # chunk width (columns of X)
SPC = NSUB // NCHUNK       # subtiles per chunk
GRP = 8                    # subtiles per output DMA group

_PROG = {}


def _patch_tile_drain():
    """This container's walrus rejects >2 sync-waits on one Drain; re-emit the
    tile epilogue drain's waits as individual wait_ge instructions."""
    import concourse.tile as tile_mod
    from concourse.tile import TileContext
    if getattr(TileContext, "_drain_patched", False):
        return

    def _patched(self, tick_clock, wait_clock):
        ScopedClock = tile_mod.ScopedClock
        nc = self.nc
        probe = nc.sync.drain()
        wait_clock.add_sem_waits(probe.ins, ScopedClock({None: tick_clock.global_clock}))
        waits = list(probe.ins.sync_info.on_wait)
        if len(waits) > 2:
            name_to_handle = {h.name: h for h in self.sems.allocated().values()}
            si = probe.ins.sync_info
            si.on_wait = []
            probe.ins.sync_info = si
            for w in waits:
                nc.sync.wait_ge(name_to_handle[w.ant_name], w.wait_value)
            nc.sync.drain()
        nc.all_engine_barrier()
        popped = nc._tile_sem_poison_stack.pop()
        assert popped is self._sem_poison
        nc.clear_and_free_semaphores(list(self.sems.allocated().values()))
        nc.all_engine_barrier()

    TileContext._drain_and_barrier = _patched
    TileContext._drain_patched = True


def _split_sync_waits(nc, limit=1):
    """walrus in this container rejects instructions carrying >2 sync waits;
    hoist the overflow onto standalone EventSemaphore instructions."""
    import concourse.mybir as mybir
    n_split = 0
    for fn in nc.m.functions:
        for bb in fn.blocks:
            new = []
            for inst in bb.instructions:
                si = inst.sync_info
                if si is not None and si.on_wait and len(si.on_wait) > limit:
                    waits = list(si.on_wait)
                    keep, over = waits[-limit:], waits[:-limit]
                    for k in range(0, len(over), limit):
                        w = mybir.InstEventSemaphore(
                            name=f"{inst.name}-w{k}",
                            engine=inst.engine,
                            ins=[], outs=[],
                            sync_info=mybir.SyncInfo(
                                on_wait=over[k:k + limit], on_update=[]),
                        )
                        new.append(w)
                        n_split += 1
                    si.on_wait = keep
                    inst.sync_info = si
                new.append(inst)
            bb.instructions = new
    return n_split


def _build_program():
    if "nc" in _PROG:
        return _PROG["nc"]
    _patch_tile_drain()
    import concourse.bass as bass
    import concourse.mybir as mybir
    from concourse.tile import TileContext

    fp32 = mybir.dt.float32
    u8 = mybir.dt.uint8
    AF = mybir.ActivationFunctionType
    OP = mybir.AluOpType
    PI_2 = float(np.pi / 2)

    nc = bass.Bass()
    t_d = nc.declare_dram_parameter("t_row", [1, RPC], fp32, isOutput=False)
    auxT_d = nc.declare_dram_parameter("auxT", [AUX + 1, RPC], fp32, isOutput=False)
    cvec_d = nc.declare_dram_parameter("cvec", [112, 1], fp32, isOutput=False)
    dvec_d = nc.declare_dram_parameter("dvec", [112, 1], fp32, isOutput=False)
    wmain_d = nc.declare_dram_parameter("wmain", [112, 512], fp32, isOutput=False)
    wr_d = nc.declare_dram_parameter("wr", [18, 4], fp32, isOutput=False)
    emb_d = nc.declare_dram_parameter("emb", [RPC, 512], fp32, isOutput=True)
    rw_d = nc.declare_dram_parameter("rw", [128, NSUB * 4], fp32, isOutput=True)
    mask_d = nc.declare_dram_parameter("mask", [128, NSUB * 4], u8, isOutput=True)

    with TileContext(nc) as tc, ExitStack() as ctx:
        const = ctx.enter_context(tc.tile_pool(name="const", bufs=1))
        xpool = ctx.enter_context(tc.tile_pool(name="x", bufs=1))
        spool = ctx.enter_context(tc.tile_pool(name="s", bufs=1))
        rpool = ctx.enter_context(tc.tile_pool(name="r", bufs=2))
        opool = ctx.enter_context(tc.tile_pool(name="o", bufs=3))
        lpsum = ctx.enter_context(tc.tile_pool(name="lp", bufs=2, space="PSUM"))
        mpsum = ctx.enter_context(tc.tile_pool(name="mp", bufs=4, space="PSUM"))

        cvec = const.tile([112, 1], fp32)
        dvec = const.tile([112, 1], fp32)
        wmain = const.tile([112, 512], fp32)
        wr_full = const.tile([82, 4], fp32)
        zro = const.tile([128, 1], fp32)
        onec = const.tile([128, 1], fp32)
        magic = const.tile([128, 1], fp32)
        nc.sync.dma_start(out=cvec[:], in_=cvec_d[:])
        nc.sync.dma_start(out=dvec[:], in_=dvec_d[:])
        nc.sync.dma_start(out=wmain[:], in_=wmain_d[:])
        nc.sync.dma_start(out=wr_full[64:82, :], in_=wr_d[:])
        nc.gpsimd.memset(zro[:], 0.0)
        nc.gpsimd.memset(onec[:], 1.0)
        nc.gpsimd.memset(magic[:], 12582912.0)  # 1.5 * 2^23: fp32 round-to-int

        X = xpool.tile([112, RPC], fp32)

        for c in range(NCHUNK):
            c0, c1 = c * CHW, (c + 1) * CHW
            # broadcast t across partitions 0:81 (feature inputs + router t row)
            nc.sync.dma_start(
                out=X[0:112, c0:c1],
                in_=t_d[0:1, c0:c1].partition_broadcast(112),
            )
            nc.sync.dma_start(out=X[65:82, c0:c1], in_=auxT_d[:, c0:c1])

            # ---- router logits (rows 96:114 only; independent of features)
            psl = lpsum.tile([128, 4 * SPC], fp32, tag="logits")
            for j in range(SPC):
                s = c * SPC + j
                nc.tensor.matmul(
                    psl[:, 4 * j:4 * j + 4],
                    lhsT=X[64:82, 128 * s:128 * (s + 1)], rhs=wr_full[64:82, :],
                    start=True, stop=True, skip_group_check=True,
                )

            S = spool.tile([112, CHW], fp32, tag="sq")
            V = spool.tile([112, CHW], fp32, tag="wv")

            # trig-table ops first, then exp-table ops (2 table sets per chunk)
            # fourier rows 0:32: y = f*t/2pi (+0.25 for cos rows); range-reduce
            # r = y - round(y) with the fp32 magic-add trick, then sin(2pi r).
            Yr = spool.tile([32, CHW], fp32, tag="yr")
            Kr = spool.tile([32, CHW], fp32, tag="kr")
            nc.scalar.activation(Yr[:, :], X[0:32, c0:c1], AF.Identity,
                                 bias=dvec[0:32], scale=cvec[0:32])
            # Kr = y + M (exact fp32 RNE on DVE); then Yr := (Kr - M) - y = -r
            nc.vector.tensor_scalar_add(Kr[:, :], Yr[:, :], 12582912.0)
            nc.vector.scalar_tensor_tensor(Yr[:, :], Kr[:, :], -12582912.0,
                                           Yr[:, :], op0=OP.add, op1=OP.subtract)
            nc.scalar.activation(X[0:32, c0:c1], Yr[:, :], AF.Sin,
                                 bias=zro[0:32], scale=float(-2 * np.pi))
            nc.scalar.activation(S[32:64, :], X[32:64, c0:c1], AF.Square,
                                 bias=dvec[32:64], scale=cvec[32:64])
            nc.scalar.activation(S[96:112, :], X[96:112, c0:c1], AF.Square,
                                 bias=dvec[96:112], scale=cvec[96:112])
            nc.scalar.activation(X[32:64, c0:c1], S[32:64, :], AF.Exp,
                                 bias=zro[32:64], scale=-1.0)
            nc.scalar.activation(X[96:112, c0:c1], S[96:112, :], AF.Exp,
                                 bias=zro[96:112], scale=-1.0)
            # wavelet amplitude (1 - u^2) = 1 - 2*q' (q' = u^2/2 in S[96:112])
            nc.scalar.activation(V[96:112, :], S[96:112, :], AF.Identity,
                                 bias=onec[96:112], scale=-2.0)
            nc.vector.tensor_mul(X[96:112, c0:c1], X[96:112, c0:c1], V[96:112, :])

            # ---- batched softmax + top-2 over [128, (j=32, e=4)]
            W2 = rpool.tile([128, 4 * SPC], fp32, tag="w2")
            Ds = rpool.tile([128, SPC], fp32, tag="ds")
            Rc = rpool.tile([128, SPC], fp32, tag="rc")
            G1 = rpool.tile([128, 4 * SPC], fp32, tag="g1")
            WL = rpool.tile([128, 4 * SPC], fp32, tag="wl")
            MF = rpool.tile([128, 4 * SPC], fp32, tag="mf")
            DSP = rpool.tile([128, 4 * SPC], fp32, tag="dsp")
            MU = rpool.tile([128, 4 * SPC], u8, tag="mu")

            w2_3 = W2[:, :].rearrange("p (j e) -> p j e", e=4)
            nc.scalar.activation(W2[:, :], psl[:, :], AF.Exp, bias=0.0, scale=1.0)
            nc.vector.reduce_sum(Ds[:, :], w2_3, axis=mybir.AxisListType.X)
            nc.vector.reciprocal(Rc[:, :], Ds[:, :])
            nc.vector.tensor_mul(w2_3, w2_3, Rc[:, :].broadcast_to((128, SPC, 4)))
            # m1 = rowmax; G1 = (w >= m1); WL = w - BIG*G1; m2 = rowmax(WL)
            nc.vector.reduce_max(Ds[:, :], w2_3, axis=mybir.AxisListType.X)
            nc.vector.tensor_tensor(G1[:, :].rearrange("p (j e) -> p j e", e=4),
                                    w2_3, Ds[:, :].broadcast_to((128, SPC, 4)),
                                    op=OP.is_ge)
            nc.vector.scalar_tensor_tensor(WL[:, :], G1[:, :], -1000.0, W2[:, :],
                                           op0=OP.mult, op1=OP.add)
            nc.vector.reduce_max(Ds[:, :], WL[:, :].rearrange("p (j e) -> p j e", e=4),
                                 axis=mybir.AxisListType.X)
            nc.vector.tensor_tensor(MF[:, :].rearrange("p (j e) -> p j e", e=4),
                                    w2_3, Ds[:, :].broadcast_to((128, SPC, 4)),
                                    op=OP.is_ge)
            nc.vector.tensor_mul(DSP[:, :], W2[:, :], MF[:, :])
            nc.vector.tensor_copy(MU[:, :], MF[:, :])

            # store raw_weights + mask for this chunk
            nc.sync.dma_start(
                out=rw_d[:, 4 * SPC * c:4 * SPC * (c + 1)], in_=W2[:, :])
            nc.sync.dma_start(
                out=mask_d[:, 4 * SPC * c:4 * SPC * (c + 1)], in_=MU[:, :])

            # ---- main expert matmuls + dispatch-scaled eviction + store
            for g in range(SPC // GRP):
                OUT = opool.tile([128, GRP * 512], fp32, tag="out")
                for jj in range(GRP):
                    j = g * GRP + jj
                    s = c * SPC + j
                    psm = mpsum.tile([128, 512], fp32, tag="mm")
                    nc.tensor.matmul(
                        psm[:, :],
                        lhsT=X[:, 128 * s:128 * (s + 1)], rhs=wmain[:],
                        start=True, stop=True,
                    )
                    nc.vector.tensor_tensor(
                        OUT[:, 512 * jj:512 * (jj + 1)].rearrange(
                            "p (e d) -> p e d", e=4),
                        psm[:, :].rearrange("p (e d) -> p e d", e=4),
                        DSP[:, 4 * j:4 * j + 4].broadcast_to((128, 4, 128)),
                        op=OP.mult,
                    )
                r0 = (c * SPC + g * GRP) * 128
                r1 = r0 + GRP * 128
                nc.sync.dma_start(
                    out=emb_d[r0:r1, :].rearrange("(j p) d -> p j d", p=128),
                    in_=OUT[:, :].rearrange("p (j d) -> p j d", d=512),
                )

    _split_sync_waits(nc)
    _PROG["nc"] = nc
    return nc


def _prep_inputs(inputs):
    ts = np.asarray(inputs["timestamp"], np.float32).reshape(B)
    aux = np.asarray(inputs["aux"], np.float32)
    router_W = np.asarray(inputs["router_W"], np.float32)
    router_b = np.asarray(inputs["router_b"], np.float32)
    freqs = np.asarray(inputs["freqs"], np.float32)
    fourier_W = np.asarray(inputs["fourier_W"], np.float32)
    knots = np.asarray(inputs["knots"], np.float32)
    spline_W = np.asarray(inputs["spline_W"], np.float32)
    centers = np.asarray(inputs["centers"], np.float32)
    gauss_W = np.asarray(inputs["gauss_W"], np.float32)
    wav_centers = np.asarray(inputs["wav_centers"], np.float32)
    wav_scales = np.asarray(inputs["wav_scales"], np.float32)
    wavelet_W = np.asarray(inputs["wavelet_W"], np.float32)

    h = 4.0 / (G - 1)
    isq2 = 1.0 / np.sqrt(2.0, dtype=np.float64)
    inv2pi = 1.0 / (2.0 * np.pi)
    cvec = np.zeros((112, 1), np.float32)
    dvec = np.zeros((112, 1), np.float32)
    cvec[0:16, 0] = freqs * inv2pi
    cvec[16:32, 0] = freqs * inv2pi
    dvec[16:32, 0] = 0.25
    cvec[32:48, 0] = 1.0 / h
    dvec[32:48, 0] = -knots / h
    cvec[48:64, 0] = 1.0
    dvec[48:64, 0] = -centers
    cvec[96:112, 0] = 1.0 / (wav_scales * np.sqrt(2.0))
    dvec[96:112, 0] = -wav_centers / (wav_scales * np.sqrt(2.0))

    wmain = np.zeros((112, 512), np.float32)
    wmain[0:32, 0:128] = fourier_W
    wmain[32:48, 128:256] = spline_W
    wmain[48:64, 256:384] = gauss_W
    wmain[96:112, 384:512] = wavelet_W
    wr = np.zeros((18, 4), np.float32)
    wr[0] = router_W[0]
    wr[1:17] = router_W[1:]
    wr[17] = router_b

    in_maps = []
    for i in range(NCORES):
        sl = slice(i * RPC, (i + 1) * RPC)
        in_maps.append({
            "t_row": np.ascontiguousarray(ts[sl]).reshape(1, RPC),
            "auxT": np.ascontiguousarray(np.vstack([aux[sl].T, np.ones((1, RPC), np.float32)])),
            "cvec": cvec, "dvec": dvec, "wmain": wmain, "wr": wr,
        })
    return in_maps


def _run(inputs, trace=False, **kw):
    from concourse import bass_utils
    nc = _build_program()
    in_maps = _prep_inputs(inputs)
    res = bass_utils.run_bass_kernel_spmd(
        nc, in_maps, core_ids=list(range(NCORES)), trace=trace, **kw)
    emb = np.concatenate([res.results[i]["emb"] for i in range(NCORES)], axis=0)

    def _untr(a, dtype):
        # [128, NSUB*4] laid out [p][s,e] -> [RPC, 4] rows b = 128*s + p
        return np.ascontiguousarray(
            a.reshape(128, NSUB, 4).transpose(1, 0, 2).reshape(RPC, 4)).astype(dtype)

    rw = np.concatenate(
        [_untr(res.results[i]["rw"], np.float32) for i in range(NCORES)], axis=0)
    mask = np.concatenate(
        [_untr(res.results[i]["mask"], np.uint8) for i in range(NCORES)],
        axis=0).astype(bool)
    return (emb, rw, mask), res


def kernel(**inputs):
    out, _ = _run(inputs, trace=False)
    return out
